# revision 9
# baseline (speedup 1.0000x reference)
"""Trainium2 Bass kernel for nn_AutoEnCode1 (dense_mlp, 8 NeuronCores).

Strategy (edge-data-parallel, per the sharding hint):
  - Shard the E=8192 edges across 8 cores (1024 edges each).
  - Host-side sharding prep (data movement / layout only, no FLOPs):
    gather Xi = A[ni], Xj = A[nj] for each core's edge slice, stack to
    Xc = [Xi; Xj] (2048 rows), ship in the layouts the TensorEngine
    needs (fp8 DoubleRow-interleaved [n, e] for mm1; bf16 [e, n] for
    the recon loss).  Weights pre-transposed + fp8, replicated.
  - Device-side compute per core (all the FLOPs):
      mm1: H1^T[h,e'] = W1^T-tiles x X^T-tiles (fp8 DoubleRow), fused
           sigmoid(+b1 per-partition bias) on ScalarE -> fp8 h1all.
      layer-1 loss: wide (H1i-H1j)^2 via VectorE, partition-reduced
           via ones-matmuls, sqrt, * label.
      mm2: H2[e,m] = H1^T-tiles (stationary) x W2^T-tiles (moving)
           into [128,2048] PSUM chunks; b2 bias folded into the same
           PSUM accumulation group via rank-1 ones x b2 matmuls;
           fused sigmoid on ScalarE into full-width [128,8192] tiles.
      layer-2 losses on full-width tiles: three VectorE subs
           (d, xi-h2i, xj-h2j), then Square+accum on ScalarE (d) and
           STT mult+accum on VectorE (recon terms), all at >=4096
           free size to amortize per-instruction overhead.
      Finalize vectorized across all edge blocks at once.
  - Host-side unshard: sum the 8x129 partials and add the
    data-independent weight regularizer loss_r.
"""

import numpy as np
import ml_dtypes

import concourse.bass as bass
import concourse.tile as tile
from concourse import bacc, mybir
from concourse.bass_utils import run_bass_kernel_spmd

N_CORES = 8
N = 8192          # node-feature dim (= num nodes)
H = 1024          # bottleneck dim
E = 8192          # num edges
PENALTY = 10.0

E_LOC = E // N_CORES      # 1024 edges per core
EP = 2 * E_LOC            # 2048 stacked rows: [Xi; Xj]

BF16 = mybir.dt.bfloat16
FP8 = mybir.dt.float8e4
F32 = mybir.dt.float32
DR = mybir.MatmulPerfMode.DoubleRow
SIG = mybir.ActivationFunctionType.Sigmoid
SQUARE = mybir.ActivationFunctionType.Square
MULT = mybir.AluOpType.mult
ADD = mybir.AluOpType.add
AXX = mybir.AxisListType.X

ts = bass.ts


def build_nc(n=N, h=H, e_loc=E_LOC, phases="ALL"):
    """Build + compile the per-core Bass graph (identical on all cores)."""
    ep = 2 * e_loc
    HT = h // 128        # h tiles
    ET = ep // 512       # e' panels of 512 (mm1 moving dim)
    EB = e_loc // 128    # edge blocks of 128 per stream ("pairs")
    EH = e_loc // 512    # e halves of 512 (layer-1 norm)
    CH = n // 2048       # phase-B psum chunks of 2048 (4 banks)

    nc = bacc.Bacc("TRN2", target_bir_lowering=False, debug=False,
                   num_devices=N_CORES)

    NT2 = n // 256       # DoubleRow contraction chunks, layer 1
    HC = h // 256        # DoubleRow contraction chunks, layer 2
    xt8 = nc.dram_tensor("xt8", [NT2, 128, 2, ep], FP8, kind="ExternalInput")
    xn = nc.dram_tensor("xn", [ep, n], BF16, kind="ExternalInput")
    w1t8 = nc.dram_tensor("w1t8", [NT2, 128, 2, h], FP8,
                          kind="ExternalInput")
    w2t8 = nc.dram_tensor("w2t8", [HC, 128, 2, n], FP8,
                          kind="ExternalInput")
    b1d = nc.dram_tensor("b1", [h], F32, kind="ExternalInput")
    b2d = nc.dram_tensor("b2", [n], BF16, kind="ExternalInput")
    labd = nc.dram_tensor("lab", [e_loc], F32, kind="ExternalInput")
    labP_d = nc.dram_tensor("labP", [128, EB], F32, kind="ExternalInput")
    facP_d = nc.dram_tensor("facP", [128, EB], F32, kind="ExternalInput")
    outd = nc.dram_tensor("out", [129], F32, kind="ExternalOutput")

    with tile.TileContext(nc) as tc:
        with (
            tc.tile_pool(name="h1", bufs=1) as h1_pool,
            tc.tile_pool(name="misc", bufs=1) as misc,
            tc.tile_pool(name="fin", bufs=1) as fin,
        ):
            # Persistent tiles
            h1all = h1_pool.tile([128, HT, ep], FP8, name="h1all",
                                 tag="h1all")
            b1t = []
            for t in range(HT):
                bt = misc.tile([128, 1], F32, name=f"b1t{t}", tag=f"b1t{t}")
                nc.sync.dma_start(bt[:], b1d.ap()[ts(t, 128)])
                b1t.append(bt)
            b2sb = misc.tile([1, n], BF16, name="b2sb", tag="b2sb")
            nc.sync.dma_start(b2sb[:], b2d.ap()[:])
            lab_f = misc.tile([1, e_loc], F32, name="labf", tag="labf")
            nc.sync.dma_start(lab_f[:], labd.ap()[:])
            labP = misc.tile([128, EB], F32, name="labP", tag="labP")
            nc.sync.dma_start(labP[:], labP_d.ap()[:])
            facP = misc.tile([128, EB], F32, name="facP", tag="facP")
            nc.sync.dma_start(facP[:], facP_d.ap()[:])
            ones_b = misc.tile([1, 128], BF16, name="ones_b", tag="ones_b")
            nc.gpsimd.memset(ones_b[:], 1.0)
            ones_col = misc.tile([128, 1], BF16, name="ones_col",
                                 tag="ones_col")
            nc.gpsimd.memset(ones_col[:], 1.0)
            l1vec = fin.tile([1, e_loc], F32, name="l1vec", tag="l1vec")

            # ---------------- Phase A: layer 1 matmul ----------------
            with (
                tc.tile_pool(name="w1", bufs=1) as w1_pool,
                tc.tile_pool(name="xa", bufs=4) as xa_pool,
                tc.tile_pool(name="psA", bufs=HT, space="PSUM") as psA,
            ):
                w1sb = [w1_pool.tile([128, 2, h], FP8, name=f"w1_{t}",
                                     tag=f"w1_{t}") for t in range(NT2)]
                for e_t in range(ET):
                    ps = [psA.tile([128, 512], F32, name="psA", tag="psA")
                          for _ in range(HT)]
                    for c in range(NT2):
                        if e_t == 0:
                            nc.sync.dma_start(w1sb[c][:], w1t8.ap()[c])
                        x = xa_pool.tile([128, 2, 512], FP8, name="x",
                                         tag="x")
                        nc.sync.dma_start(x[:], xt8.ap()[c][:, :,
                                                           ts(e_t, 512)])
                        for h_t in range(HT):
                            nc.tensor.matmul(ps[h_t][:],
                                             w1sb[c][:, :, ts(h_t, 128)],
                                             x[:],
                                             start=(c == 0),
                                             stop=(c == NT2 - 1),
                                             perf_mode=DR)
                    for h_t in range(HT):
                        nc.scalar.activation(h1all[:, h_t, ts(e_t, 512)],
                                             ps[h_t][:], SIG,
                                             bias=b1t[h_t][:])

            if phases == "A":
                dumm = fin.tile([1, 1], F32, name="dumm", tag="dumm")
                nc.scalar.activation(dumm[:], h1all[0:1, 0, 0:1],
                                     mybir.ActivationFunctionType.Identity)
                nc.sync.dma_start(outd.ap()[0:1], dumm[0:1, 0:1])

            # ---------------- Phase A2: layer-1 diff loss ----------------
            if phases != "A":
                with (
                    tc.tile_pool(name="l1s", bufs=2) as l1s,
                    tc.tile_pool(name="psS", bufs=2, space="PSUM") as psS,
                ):
                    # sqrt(sum_h (H1i-H1j)^2) * lab; wide ops over all HT
                    for eh in range(EH):
                        dw = l1s.tile([128, HT, 512], BF16, name="dw",
                                      tag="dw")
                        nc.vector.tensor_sub(
                            dw[:],
                            h1all[:, :, eh * 512:(eh + 1) * 512],
                            h1all[:, :, e_loc + eh * 512:
                                  e_loc + (eh + 1) * 512])
                        d2w = l1s.tile([128, HT, 512], BF16, name="d2w",
                                       tag="d2w")
                        nc.vector.scalar_tensor_tensor(
                            d2w[:], dw[:], 0.0, dw[:], ADD, MULT)
                        l1ps = psS.tile([1, 512], F32, name="l1ps",
                                        tag="l1ps")
                        for h_t in range(HT):
                            nc.tensor.matmul(l1ps[:], ones_col[:],
                                             d2w[:, h_t],
                                             start=(h_t == 0),
                                             stop=(h_t == HT - 1))
                        l1n = l1s.tile([1, 512], F32, name="l1n", tag="l1n",
                                       bufs=1)
                        nc.scalar.sqrt(l1n[:], l1ps[:])
                        nc.vector.tensor_mul(
                            l1vec[:, eh * 512:(eh + 1) * 512], l1n[:],
                            lab_f[:, eh * 512:(eh + 1) * 512])
                    l1sc = fin.tile([1, 1], F32, name="l1sc", tag="l1sc")
                    nc.vector.reduce_sum(l1sc[:], l1vec[:], axis=AXX)

                if phases == "A2":
                    nc.sync.dma_start(outd.ap()[128:129], l1sc[0:1, 0:1])

                # ---------------- Phase B: layer 2 ----------------
                if phases != "A2":
                  with (
                    tc.tile_pool(name="w2", bufs=1) as w2_pool,
                    tc.tile_pool(name="h2", bufs=6) as h2_pool,
                    tc.tile_pool(name="dd", bufs=4) as dd_pool,
                    tc.tile_pool(name="acc", bufs=1) as acc_pool,
                    tc.tile_pool(name="psB", bufs=2, space="PSUM") as psB,
                  ):
                    CH2 = n // 1024
                    w2sb = [w2_pool.tile([128, 2, n], FP8, name=f"w2_{t}",
                                         tag=f"w2_{t}") for t in range(HC)]
                    junk = misc.tile([128, 2048], BF16, name="junk",
                                     tag="junk")
                    junkv = misc.tile([128, 2048], BF16, name="junkv",
                                      tag="junkv")
                    accd = acc_pool.tile([128, EB, 4], F32, name="accd",
                                         tag="accd")
                    accri = acc_pool.tile([128, EB, 4], F32, name="accri",
                                          tag="accri")
                    accrj = acc_pool.tile([128, EB, 4], F32, name="accrj",
                                          tag="accrj")

                    for p in range(EB):
                        for c2 in range(4):
                            h2i = h2_pool.tile([128, 2048], BF16,
                                               name="h2i", tag="h2i")
                            h2j = h2_pool.tile([128, 2048], BF16,
                                               name="h2j", tag="h2j")
                            for sub in range(2):
                                ch = 2 * c2 + sub
                                psi = psB.tile([128, 1024], F32, name="psi",
                                               tag="psi")
                                psj = psB.tile([128, 1024], F32, name="psj",
                                               tag="psj")
                                for pst in (psi, psj):
                                    for s in range(2):
                                        nc.tensor.matmul(
                                            pst[:, ts(s, 512)], ones_b[:],
                                            b2sb[:, ch * 1024 + s * 512:
                                                 ch * 1024 + (s + 1) * 512],
                                            start=True, stop=False)
                                for cc in range(HC):
                                    if p == 0 and ch == 0:
                                        nc.sync.dma_start(w2sb[cc][:],
                                                          w2t8.ap()[cc])
                                    for s in range(2):
                                        nc.tensor.matmul(
                                            psi[:, ts(s, 512)],
                                            h1all[:, 2 * cc:2 * cc + 2,
                                                  ts(p, 128)],
                                            w2sb[cc][:, :,
                                                     ch * 1024 + s * 512:
                                                     ch * 1024 +
                                                     (s + 1) * 512],
                                            start=False,
                                            stop=(cc == HC - 1),
                                            perf_mode=DR)
                                for cc in range(HC):
                                    for s in range(2):
                                        nc.tensor.matmul(
                                            psj[:, ts(s, 512)],
                                            h1all[:, 2 * cc:2 * cc + 2,
                                                  ts(EB + p, 128)],
                                            w2sb[cc][:, :,
                                                     ch * 1024 + s * 512:
                                                     ch * 1024 +
                                                     (s + 1) * 512],
                                            start=False,
                                            stop=(cc == HC - 1),
                                            perf_mode=DR)
                                nc.scalar.activation(h2i[:, ts(sub, 1024)],
                                                     psi[:], SIG)
                                nc.scalar.activation(h2j[:, ts(sub, 1024)],
                                                     psj[:], SIG)

                            sl2 = slice(c2 * 2048, (c2 + 1) * 2048)
                            d = dd_pool.tile([128, 2048], BF16, name="dB",
                                             tag="dB")
                            nc.vector.tensor_sub(d[:], h2i[:], h2j[:])
                            nc.scalar.activation(
                                junk[:], d[:], SQUARE,
                                accum_out=accd[:, p, c2:c2 + 1])
                            nc.gpsimd.dma_start(
                                h2i[:], xn.ap()[ts(p, 128), sl2],
                                accum_op=ADD)
                            nc.vector.scalar_tensor_tensor(
                                junkv[:], h2i[:], 0.0, h2i[:], ADD, MULT,
                                accum_out=accri[:, p, c2:c2 + 1])
                            nc.gpsimd.dma_start(
                                h2j[:], xn.ap()[ts(EB + p, 128), sl2],
                                accum_op=ADD)
                            nc.vector.scalar_tensor_tensor(
                                junkv[:], h2j[:], 0.0, h2j[:], ADD, MULT,
                                accum_out=accrj[:, p, c2:c2 + 1])

                    # -------- vectorized finalize over all p --------
                    sd = fin.tile([128, EB], F32, name="sd", tag="sd")
                    nc.vector.tensor_reduce(sd[:], accd[:], AXX, ADD)
                    sri = fin.tile([128, EB], F32, name="sri", tag="sri")
                    nc.vector.tensor_reduce(sri[:], accri[:], AXX, ADD)
                    srj = fin.tile([128, EB], F32, name="srj", tag="srj")
                    nc.vector.tensor_reduce(srj[:], accrj[:], AXX, ADD)
                    nd = fin.tile([128, EB], F32, name="nd", tag="nd")
                    nc.scalar.sqrt(nd[:], sd[:])
                    nri = fin.tile([128, EB], F32, name="nri", tag="nri")
                    nc.scalar.sqrt(nri[:], sri[:])
                    nrj = fin.tile([128, EB], F32, name="nrj", tag="nrj")
                    nc.scalar.sqrt(nrj[:], srj[:])
                    t1 = fin.tile([128, EB], F32, name="t1", tag="t1")
                    nc.vector.tensor_mul(t1[:], nd[:], labP[:])
                    t2 = fin.tile([128, EB], F32, name="t2", tag="t2")
                    nc.vector.tensor_add(t2[:], nri[:], nrj[:])
                    t3 = fin.tile([128, EB], F32, name="t3", tag="t3")
                    nc.vector.tensor_mul(t3[:], t2[:], facP[:])
                    pacc = fin.tile([128, EB], F32, name="pacc", tag="pacc")
                    nc.vector.tensor_add(pacc[:], t1[:], t3[:])
                    pv = fin.tile([128, 1], F32, name="pv", tag="pv")
                    nc.vector.reduce_sum(pv[:], pacc[:], axis=AXX)
                    nc.sync.dma_start(outd.ap()[0:128], pv[:, 0:1])
                    nc.sync.dma_start(outd.ap()[128:129], l1sc[0:1, 0:1])

    nc.compile()
    return nc


_NC_CACHE = {}


def _get_nc():
    if "nc" not in _NC_CACHE:
        _NC_CACHE["nc"] = build_nc()
    return _NC_CACHE["nc"]


def make_in_maps(A, W1, b1, W2, b2, edges, labels):
    bf16 = ml_dtypes.bfloat16
    ni = edges[:, 0].astype(np.int64)
    nj = edges[:, 1].astype(np.int64)

    fp8 = ml_dtypes.float8_e4m3
    nn, hh = W1.shape[1], W1.shape[0]
    # interleaved DoubleRow layouts: [chunk, p, plane, out-dim] where
    # contraction row k = 256*chunk + 128*plane + p
    W1T8 = np.ascontiguousarray(
        W1.T.reshape(nn // 256, 2, 128, hh).transpose(0, 2, 1, 3)
    ).astype(fp8)
    W2T8 = np.ascontiguousarray(
        W2.T.reshape(hh // 256, 2, 128, nn).transpose(0, 2, 1, 3)
    ).astype(fp8)
    b1f = b1.astype(np.float32)
    b2bf = b2.astype(bf16)

    in_maps = []
    for c in range(N_CORES):
        sl = slice(c * E_LOC, (c + 1) * E_LOC)
        Xc = np.concatenate([A[ni[sl]], A[nj[sl]]], axis=0)   # [EP, N] f32
        Xcb = (-Xc).astype(bf16)
        XT8 = np.ascontiguousarray(
            Xc.T.reshape(nn // 256, 2, 128, Xc.shape[0]).transpose(0, 2, 1, 3)
        ).astype(fp8)
        lab = labels[sl].astype(np.float32)
        fac = np.where(lab >= 1.0, np.float32(PENALTY),
                       np.float32(1.0)).astype(np.float32)
        labP = np.ascontiguousarray(lab.reshape(E_LOC // 128, 128).T)
        facP = np.ascontiguousarray(fac.reshape(E_LOC // 128, 128).T)
        in_maps.append({
            "xt8": XT8, "xn": Xcb, "w1t8": W1T8, "w2t8": W2T8,
            "b1": b1f, "b2": b2bf, "lab": lab, "labP": labP, "facP": facP,
        })
    return in_maps


def host_loss_r(W1, b1, W2, b2):
    # Weight regularizer: data-independent constant (per-edge, per-layer
    # sum of weight row norms + bias norm, times E).
    return float(E) * (
        np.linalg.norm(W1, axis=1).sum() + np.linalg.norm(b1)
        + np.linalg.norm(W2, axis=1).sum() + np.linalg.norm(b2)
    )


def kernel(A, W1, b1, W2, b2, edges, labels):
    A = np.asarray(A, dtype=np.float32)
    W1 = np.asarray(W1, dtype=np.float32)
    b1 = np.asarray(b1, dtype=np.float32)
    W2 = np.asarray(W2, dtype=np.float32)
    b2 = np.asarray(b2, dtype=np.float32)
    edges = np.asarray(edges)
    labels = np.asarray(labels)

    in_maps = make_in_maps(A, W1, b1, W2, b2, edges, labels)
    nc = _get_nc()
    res = run_bass_kernel_spmd(nc, in_maps, core_ids=list(range(N_CORES)))
    part = sum(float(res.results[c]["out"].sum()) for c in range(N_CORES))
    return np.array(part + host_loss_r(W1, b1, W2, b2), dtype=np.float32)
